# revision 1
# baseline (speedup 1.0000x reference)
"""CornerPool module kernel for Trainium2 (Bass/Tile), 8-core batch-parallel.

Model (per sample, C=256, H=W=128):
  t = relu(bn(conv3x3(x, w_t)));  tp = reverse-cummax_H(t)
  l = relu(bn(conv3x3(x, w_l)));  lp = reverse-cummax_W(l)
  b = relu(bn(conv3x3(x, w_b)));  bp = cummax_H(b)
  r = relu(bn(conv3x3(x, w_r)));  rp = cummax_W(r)
  tl = relu(bn3(conv3x3(tp+lp)) + bn1(conv1x1(x)));  out_tl = relu(bn(conv3x3(tl)))
  br = relu(bn3(conv3x3(bp+rp)) + bn1(conv1x1(x)));  out_br = relu(bn(conv3x3(br)))

Strategy: one sample per NeuronCore (B=8). All convs lowered to f32r
(full-rate fp32) matmuls over 128-channel tiles with N=512 (4 image rows)
PSUM accumulation groups; BN scale folded into weights on host, bias applied
in the ScalarE relu epilogue. Corner pools: H-direction via 2-step
shifted-max doubling + inter-strip carry, W-direction via the native DVE
prefix-scan instruction (per image row, reversed AP for left-pool).
Intermediates (pooled maps, tp+lp sums, tl/br) round-trip through padded
internal-DRAM scratch so every 3x3 conv reads zero-padded halos uniformly.
"""

import numpy as np

_P = 128          # partitions / channel tile
_SR = 4           # image rows per strip (N = _SR*128 = 512)


def _prep_host(inputs):
    """Fold BN scales into weights, build lhsT-layout weight arrays and the
    combined bias table. Returns dict of shared input arrays."""
    f32 = np.float32

    def scaled(name):
        w = np.asarray(inputs["w_" + name], f32)
        s = np.asarray(inputs["s_" + name], f32)
        return w * s[:, None, None, None]

    def bias(name):
        return np.asarray(inputs["b_" + name], f32)

    # stage A convs: [128co, 256ci, 3, 3] -> [128k, 18(ci_t*9+dydx), 128m]
    def layA(w):
        a = w.transpose(1, 2, 3, 0).reshape(2, 128, 9, 128)   # ci_t,k,dydx,m
        return np.ascontiguousarray(a.transpose(1, 0, 2, 3).reshape(128, 18, 128))

    wa = np.stack([layA(scaled(n)) for n in ("t", "l", "b", "r")])  # [4,128,18,128]

    # stage C: w3 [256co,128ci,3,3] -> [k, co_t*9+dydx, m];
    #          w1 [256co,256ci,1,1] -> [k, co_t*2+ci_t, m]; concat -> 22 slices
    def layC(w3, w1):
        a3 = w3.transpose(1, 2, 3, 0).reshape(128, 9, 2, 128)     # k,dydx,co_t,m
        a3 = a3.transpose(0, 2, 1, 3).reshape(128, 18, 128)
        a1 = w1[:, :, 0, 0].T.reshape(2, 128, 2, 128)             # ci_t,k,co_t,m
        a1 = a1.transpose(1, 2, 0, 3).reshape(128, 4, 128)        # k, co_t*2+ci_t, m
        return np.ascontiguousarray(np.concatenate([a3, a1], axis=1))

    wc = np.stack([layC(scaled("tl3"), scaled("tl1")),
                   layC(scaled("br3"), scaled("br1"))])            # [2,128,22,128]

    # stage D: [256co,256ci,3,3] -> [k, co_t, ci_t*9+dydx, m]
    def layD(w):
        a = w.transpose(1, 2, 3, 0).reshape(2, 128, 3, 3, 2, 128)  # ci_t,k,dy,dx,co_t,m
        a = a.transpose(1, 4, 0, 2, 3, 5).reshape(128, 2, 18, 128)
        return np.ascontiguousarray(a)

    wd = np.stack([layD(scaled("tlo")), layD(scaled("bro"))])      # [2,128,2,18,128]

    bias_rows = [bias("t"), bias("l"), bias("b"), bias("r")]       # 0..3
    for bi, (n3, n1) in enumerate((("tl3", "tl1"), ("br3", "br1"))):
        comb = bias(n3) + bias(n1)                                 # [256]
        bias_rows += [comb[:128], comb[128:]]                      # 4+bi*2+co_t
    for n in ("tlo", "bro"):
        bb = bias(n)
        bias_rows += [bb[:128], bb[128:]]                          # 8+bi*2+co_t
    bias_all = np.ascontiguousarray(np.stack(bias_rows).T).astype(f32)  # [128,12]

    return {"wa": wa, "wc": wc, "wd": wd, "bias": bias_all}


def _pad_x_sample(xs, H):
    """[256,H,128] f32 -> [2,128,H+2,130] zero-padded."""
    xp = np.zeros((2, 128, H + 2, 130), np.float32)
    xp[:, :, 1:H + 1, 1:129] = xs.reshape(2, 128, H, 128)
    return xp


def _build(H):
    """Build the Bass module for one core (one sample of height H)."""
    import concourse.bacc as bacc
    import concourse.mybir as mybir
    import concourse.tile as tile

    dt = mybir.dt
    Alu = mybir.AluOpType
    Act = mybir.ActivationFunctionType
    S = H // _SR
    HP = H + 2
    NPIX = HP * 130

    nc = bacc.Bacc("TRN2", target_bir_lowering=False, debug=False)

    xpad = nc.dram_tensor("xpad", [2, 128, HP, 130], dt.float32, kind="ExternalInput")
    wa_d = nc.dram_tensor("wa", [4, 128, 18, 128], dt.float32, kind="ExternalInput")
    wc_d = nc.dram_tensor("wc", [2, 128, 22, 128], dt.float32, kind="ExternalInput")
    wd_d = nc.dram_tensor("wd", [2, 128, 2, 18, 128], dt.float32, kind="ExternalInput")
    bias_d = nc.dram_tensor("bias", [128, 12], dt.float32, kind="ExternalInput")
    out_tl = nc.dram_tensor("out_tl", [256, H, 128], dt.float32, kind="ExternalOutput")
    out_br = nc.dram_tensor("out_br", [256, H, 128], dt.float32, kind="ExternalOutput")

    # internal DRAM scratch (f32r, produced rounded on-chip)
    tp_d = nc.dram_tensor("tp_s", [128, H, 128], dt.float32r)
    bp_d = nc.dram_tensor("bp_s", [128, H, 128], dt.float32r)
    sum_d = nc.dram_tensor("sum_s", [2, 128, HP, 130], dt.float32r)
    tlb_d = nc.dram_tensor("tlb_s", [2, 2, 128, HP, 130], dt.float32r)

    with tile.TileContext(nc) as tc:
        import contextlib
        with contextlib.ExitStack() as ctx:
            xpool = ctx.enter_context(tc.tile_pool(name="xp", bufs=1))
            wpool = ctx.enter_context(tc.tile_pool(name="wp", bufs=3))
            spool = ctx.enter_context(tc.tile_pool(name="sp", bufs=2))
            wpool2 = ctx.enter_context(tc.tile_pool(name="wide", bufs=3))
            hpool = ctx.enter_context(tc.tile_pool(name="hp", bufs=3))
            cpool = ctx.enter_context(tc.tile_pool(name="cp", bufs=2))
            mpool = ctx.enter_context(tc.tile_pool(name="mp", bufs=1))
            pspool = ctx.enter_context(tc.tile_pool(name="ps", bufs=8, space="PSUM"))

            # ---- preamble: x, biases, zero borders --------------------
            nch = 4
            bounds = [HP - (HP * k) // nch for k in range(nch + 1)]  # desc
            xt0 = xpool.tile([128, NPIX], dt.float32r, tag="x0")
            xt1 = xpool.tile([128, NPIX], dt.float32r, tag="x1")
            xt = [xt0, xt1]

            def load_x_chunk(k):
                for ci, eng in ((0, nc.sync), (1, nc.scalar)):
                    a, b = bounds[k + 1], bounds[k]
                    seg = xt[ci][:, a * 130:b * 130]
                    eng.dma_start(seg,
                                  xpad.ap()[ci][:, a:b, :].bitcast(dt.float32r))
                    nc.vector.tensor_copy(seg, seg.bitcast(dt.float32))

            load_x_chunk(0)
            xr = [t[:].rearrange("p (a b) -> p a b", b=130) for t in xt]

            bt = mpool.tile([128, 12], dt.float32, tag="bias")
            nc.sync.dma_start(bt[:], bias_d.ap())


            def load_w(src_ap, nsl):
                t = wpool.tile([128, nsl, 128], dt.float32r, tag="w")
                h = nsl // 2
                r = src_ap.bitcast(dt.float32r)
                nc.sync.dma_start(t[:, :h], r[:, :h])
                nc.scalar.dma_start(t[:, h:], r[:, h:])
                nc.vector.tensor_copy(t[:], t[:].bitcast(dt.float32))
                return t

            def conv_a_mms(ps, w, s):
                i = 0
                for ci in range(2):
                    for dy in range(3):
                        for dx in range(3):
                            nc.tensor.matmul(
                                ps[:], w[:, ci * 9 + dy * 3 + dx],
                                xr[ci][:, _SR * s + dy:_SR * s + dy + _SR,
                                       dx:dx + 128],
                                start=(i == 0), stop=(i == 17))
                            i += 1

            def act_strip(ps, brow, dtype=dt.float32r):
                t = spool.tile([128, _SR, 128], dtype, tag="ct")
                nc.scalar.activation(t[:].rearrange("p a b -> p (a b)"), ps[:],
                                     Act.Relu, bias=bt[:, brow:brow + 1],
                                     scale=1.0)
                return t

            def act_strip_wide(ps, brow):
                # [128, 4, 130] with zeroed w-border columns; ACT fills interior
                t = wpool2.tile([128, _SR, 130], dt.float32r, tag="cw")
                nc.gpsimd.memset(t[:, :, 0:1].bitcast(dt.float32), 0.0)
                nc.gpsimd.memset(t[:, :, 129:130].bitcast(dt.float32), 0.0)
                nc.scalar.activation(t[:, :, 1:129], ps[:],
                                     Act.Relu, bias=bt[:, brow:brow + 1],
                                     scale=1.0)
                return t

            # ---- pass T: conv t, reverse cummax over H (strips desc) --
            w_t = load_w(wa_d.ap()[0], 18)
            for _k in range(1, nch):
                load_x_chunk(_k)
            zt = mpool.tile([128, 130], dt.float32r, tag="zero")
            nc.vector.memset(zt[:].bitcast(dt.float32), 0.0)
            for i, buf in enumerate((sum_d.ap()[0], sum_d.ap()[1],
                                     tlb_d.ap()[0, 0], tlb_d.ap()[0, 1],
                                     tlb_d.ap()[1, 0], tlb_d.ap()[1, 1])):
                eng = nc.sync if i % 2 else nc.scalar
                eng.dma_start(buf[:, 0, :], zt[:, :130])
                eng.dma_start(buf[:, HP - 1, :], zt[:, :130])

            carry = cpool.tile([128, 1, 128], dt.float32r, tag="cryT")
            nc.vector.memset(carry[:].bitcast(dt.float32), 0.0)
            for s in reversed(range(S)):
                ps = pspool.tile([128, 512], dt.float32, tag="ps")
                conv_a_mms(ps, w_t, s)
                ct = act_strip(ps, 0)
                nc.vector.tensor_tensor(ct[:, 0:3], ct[:, 0:3], ct[:, 1:4], Alu.max)
                nc.vector.tensor_tensor(ct[:, 0:2], ct[:, 0:2], ct[:, 2:4], Alu.max)
                nc.vector.tensor_tensor(ct[:], ct[:],
                                        carry[:].broadcast_to([128, _SR, 128]),
                                        Alu.max)
                if s != 0:
                    nxt = cpool.tile([128, 1, 128], dt.float32r, tag="cryT")
                    nc.vector.tensor_copy(nxt[:], ct[:, 0:1])
                    carry = nxt
                nc.sync.dma_start(tp_d.ap()[:, _SR * s:_SR * (s + 1), :], ct[:])

            # ---- pass B: conv b, forward cummax over H (asc) ----------
            w_b = load_w(wa_d.ap()[2], 18)
            carry = cpool.tile([128, 1, 128], dt.float32r, tag="cryB")
            nc.vector.memset(carry[:].bitcast(dt.float32), 0.0)
            for s in range(S):
                ps = pspool.tile([128, 512], dt.float32, tag="ps")
                conv_a_mms(ps, w_b, s)
                ct = act_strip(ps, 2)
                p1 = spool.tile([128, _SR, 128], dt.float32r, tag="p1")
                nc.vector.tensor_tensor(p1[:, 1:4], ct[:, 1:4], ct[:, 0:3], Alu.max)
                nc.vector.tensor_copy(p1[:, 0:1], ct[:, 0:1])
                nc.vector.tensor_tensor(p1[:, 2:4], p1[:, 2:4], p1[:, 0:2], Alu.max)
                nc.vector.tensor_tensor(p1[:], p1[:],
                                        carry[:].broadcast_to([128, _SR, 128]),
                                        Alu.max)
                if s != S - 1:
                    nxt = cpool.tile([128, 1, 128], dt.float32r, tag="cryB")
                    nc.vector.tensor_copy(nxt[:], p1[:, 3:4])
                    carry = nxt
                nc.sync.dma_start(bp_d.ap()[:, _SR * s:_SR * (s + 1), :], p1[:])

            # ---- pass L: conv l, reverse cummax over W, add tp --------
            w_l = load_w(wa_d.ap()[1], 18)
            for s in range(S):
                ps = pspool.tile([128, 512], dt.float32, tag="ps")
                conv_a_mms(ps, w_l, s)
                ct = act_strip_wide(ps, 1)
                for h in range(_SR):
                    v = ct[:, h, 1:129][:, ::-1]
                    nc.vector.tensor_tensor_scan(v, v, v, 0.0,
                                                 op0=Alu.max, op1=Alu.bypass)
                tps = spool.tile([128, _SR, 128], dt.float32r, tag="tps")
                nc.sync.dma_start(tps[:], tp_d.ap()[:, _SR * s:_SR * (s + 1), :])
                nc.vector.tensor_tensor(ct[:, :, 1:129], ct[:, :, 1:129],
                                        tps[:], Alu.add)
                nc.sync.dma_start(
                    sum_d.ap()[0][:, 1 + _SR * s:1 + _SR * (s + 1), :], ct[:])

            # ---- pass R: conv r, forward cummax over W, add bp --------
            w_r = load_w(wa_d.ap()[3], 18)
            for s in range(S):
                ps = pspool.tile([128, 512], dt.float32, tag="ps")
                conv_a_mms(ps, w_r, s)
                ct = act_strip_wide(ps, 3)
                for h in range(_SR):
                    v = ct[:, h, 1:129]
                    nc.vector.tensor_tensor_scan(v, v, v, 0.0,
                                                 op0=Alu.max, op1=Alu.bypass)
                tps = spool.tile([128, _SR, 128], dt.float32r, tag="tps")
                nc.sync.dma_start(tps[:], bp_d.ap()[:, _SR * s:_SR * (s + 1), :])
                nc.vector.tensor_tensor(ct[:, :, 1:129], ct[:, :, 1:129],
                                        tps[:], Alu.add)
                nc.sync.dma_start(
                    sum_d.ap()[1][:, 1 + _SR * s:1 + _SR * (s + 1), :], ct[:])

            # ---- stage C: tl = relu(conv3x3(sum) + conv1x1(x)) --------
            for bi in range(2):
                w_c = load_w(wc_d.ap()[bi], 22)
                for s in range(S):
                    sums = hpool.tile([128, 6, 130], dt.float32r, tag="sums")
                    nc.sync.dma_start(sums[:],
                                      sum_d.ap()[bi][:, _SR * s:_SR * s + 6, :])
                    for co in range(2):
                        ps = pspool.tile([128, 512], dt.float32, tag="ps")
                        i = 0
                        for dy in range(3):
                            for dx in range(3):
                                nc.tensor.matmul(
                                    ps[:], w_c[:, co * 9 + dy * 3 + dx],
                                    sums[:, dy:dy + _SR, dx:dx + 128],
                                    start=(i == 0), stop=False)
                                i += 1
                        for ci in range(2):
                            nc.tensor.matmul(
                                ps[:], w_c[:, 18 + co * 2 + ci],
                                xr[ci][:, 1 + _SR * s:1 + _SR * (s + 1), 1:129],
                                start=False, stop=(ci == 1))
                        cst = act_strip_wide(ps, 4 + bi * 2 + co)
                        nc.sync.dma_start(
                            tlb_d.ap()[bi, co][:, 1 + _SR * s:1 + _SR * (s + 1),
                                               :], cst[:])

            # ---- stage D: out = relu(conv3x3(tl)) ---------------------
            for bi in range(2):
                wd0 = load_w(wd_d.ap()[bi, :, 0], 18)
                wd1 = load_w(wd_d.ap()[bi, :, 1], 18)
                out_d = out_tl if bi == 0 else out_br
                for s in range(S):
                    din = []
                    for ci in range(2):
                        t = hpool.tile([128, 6, 130], dt.float32r, tag="dls")
                        nc.sync.dma_start(
                            t[:], tlb_d.ap()[bi, ci][:, _SR * s:_SR * s + 6, :])
                        din.append(t)
                    for co, w in ((0, wd0), (1, wd1)):
                        ps = pspool.tile([128, 512], dt.float32, tag="ps")
                        i = 0
                        for ci in range(2):
                            for dy in range(3):
                                for dx in range(3):
                                    nc.tensor.matmul(
                                        ps[:], w[:, ci * 9 + dy * 3 + dx],
                                        din[ci][:, dy:dy + _SR, dx:dx + 128],
                                        start=(i == 0), stop=(i == 17))
                                    i += 1
                        ot = act_strip(ps, 8 + bi * 2 + co, dtype=dt.float32)
                        nc.sync.dma_start(
                            out_d.ap()[co * 128:(co + 1) * 128,
                                       _SR * s:_SR * (s + 1), :], ot[:])

    nc.compile()
    return nc


_NC_CACHE = {}


def _get_nc(H):
    if H not in _NC_CACHE:
        _NC_CACHE[H] = _build(H)
    return _NC_CACHE[H]


def kernel(**inputs):
    from concourse import bass_utils

    x = np.asarray(inputs["x"], np.float32)
    B, C, H, W = x.shape
    assert (C, W) == (256, 128) and H % _SR == 0

    shared = _prep_host(inputs)
    nc = _get_nc(H)

    in_maps = []
    for b in range(B):
        m = dict(shared)
        m["xpad"] = _pad_x_sample(x[b], H)
        in_maps.append(m)

    import os
    trace = bool(int(os.environ.get("KERNEL_TRACE", "0")))
    res = bass_utils.run_bass_kernel_spmd(
        nc, in_maps, core_ids=list(range(B)), trace=trace)
    kernel.last_result = res

    otl = np.stack([res.results[b]["out_tl"].reshape(256, H, 128)
                    for b in range(B)])
    obr = np.stack([res.results[b]["out_br"].reshape(256, H, 128)
                    for b in range(B)])
    return otl, obr



# revision 2
# speedup vs baseline: 1.0007x; 1.0007x over previous
"""CornerPool kernel for Trainium2 — fused 1D Winograd F(2,3) along H, bf16.

One sample per NeuronCore (B=8). All 3x3 convs use Winograd F(2,3) on the
H axis (2 output rows per tile, taps along W stay direct): per output
chunk of 8 rows, 4 PSUM banks accumulate M_u = sum_{ci,dx} U_u^T V_u with
U_u = G-transformed (BN-folded) weights; DVE combines y_even=M0+M1+M2,
y_odd=M1-M2-M3; ScalarE applies bias+ReLU. The 1x1 convs of stage C are
folded into the M0 (+w1) and M3 (-w1) accumulations, so they ride the
same inverse. Corner pools: H pools via shifted-max doubling (GpSimd) +
carry; W pools via DVE prefix-scan per row. The whole net runs fused in
SBUF (two directional passes: TL descending, BR ascending) — x, weights
and rolling sum/tl windows stay on-chip; only x/weights in and outputs
out touch DRAM.
"""

import numpy as np

_P = 128
_CH = 16          # chunks per image; chunk = 8 image rows = 4 Winograd tiles
_G = np.array([[1, 0, 0], [0.5, 0.5, 0.5], [0.5, -0.5, 0.5], [0, 0, 1]],
              np.float32)


def _bf16():
    import ml_dtypes
    return ml_dtypes.bfloat16


def _prep_host(inputs):
    """Fold BN scales, G-transform weights along dy, build bf16 lhsT arrays."""
    f32 = np.float32
    BF = _bf16()

    def scaled(name):
        w = np.asarray(inputs["w_" + name], f32)
        s = np.asarray(inputs["s_" + name], f32)
        return w * s[:, None, None, None]

    def bias(name):
        return np.asarray(inputs["b_" + name], f32)

    def gtrans(w):
        # w [co, ci, 3, 3] -> [ci, 4u, 3dx, co]
        return np.einsum('uy,oiyx->iuxo', _G, w).astype(f32)

    # stage A: [ci=256, 4, 3, co=128] -> [4conv][128k, 2ci*12, 128m]
    def layA(w):
        a = gtrans(w).reshape(2, 128, 12, 128)
        return np.ascontiguousarray(a.transpose(1, 0, 2, 3).reshape(128, 24, 128))

    ua = np.stack([layA(scaled(n)) for n in ("t", "l", "b", "r")]).astype(BF)

    # stage C3: [ci=128, 4, 3, co=256] -> [2br][128k, 2co*12, 128m]
    def layC(w3):
        a = gtrans(w3).reshape(128, 12, 2, 128)
        return np.ascontiguousarray(a.transpose(0, 2, 1, 3).reshape(128, 24, 128))

    uc = np.stack([layC(scaled("tl3")), layC(scaled("br3"))]).astype(BF)

    # stage C1: [co=256, ci=256] -> [2br][128k, co_t*4 + sign*2 + ci_t, 128m]
    def layC1(w1):
        a = w1[:, :, 0, 0].T.reshape(2, 128, 2, 128)   # ci_t, k, co_t, m
        both = np.stack([a, -a], axis=0)               # sign, ci_t, k, co_t, m
        return np.ascontiguousarray(
            both.transpose(2, 3, 0, 1, 4).reshape(128, 8, 128))

    w1 = np.stack([layC1(scaled("tl1")), layC1(scaled("br1"))]).astype(BF)

    # stage D: [ci=256, 4, 3, co=256] -> [2br][128k, co_t*24 + ci_t*12 + uxdx, 128m]
    def layD(w):
        a = gtrans(w).reshape(2, 128, 12, 2, 128)      # ci_t, k, uxdx, co_t, m
        return np.ascontiguousarray(
            a.transpose(1, 3, 0, 2, 4).reshape(128, 48, 128))

    ud = np.stack([layD(scaled("tlo")), layD(scaled("bro"))]).astype(BF)

    bias_rows = [bias("t"), bias("l"), bias("b"), bias("r")]
    for n3, n1 in (("tl3", "tl1"), ("br3", "br1")):
        comb = bias(n3) + bias(n1)
        bias_rows += [comb[:128], comb[128:]]
    for n in ("tlo", "bro"):
        bb = bias(n)
        bias_rows += [bb[:128], bb[128:]]
    bias_all = np.ascontiguousarray(np.stack(bias_rows).T).astype(f32)

    return {"ua": ua, "uc": uc, "w1": w1, "ud": ud, "bias": bias_all}


def _pad_x_sample(xs, H):
    """[256, H, 128] f32 -> [2, 128, H+2, 130] bf16 zero-padded."""
    BF = _bf16()
    xp = np.zeros((2, 128, H + 2, 130), BF)
    xp[:, :, 1:H + 1, 1:129] = xs.reshape(2, 128, H, 128).astype(BF)
    return xp


def _build(H):
    import concourse.bacc as bacc
    import concourse.mybir as mybir
    import concourse.tile as tile
    import contextlib

    dt = mybir.dt
    Alu = mybir.AluOpType
    Act = mybir.ActivationFunctionType
    BF = dt.bfloat16
    CH = H // 8
    HP = H + 2

    nc = bacc.Bacc("TRN2", target_bir_lowering=False, debug=False)

    xpad = nc.dram_tensor("xpad", [2, 128, HP, 130], BF, kind="ExternalInput")
    ua_d = nc.dram_tensor("ua", [4, 128, 24, 128], BF, kind="ExternalInput")
    uc_d = nc.dram_tensor("uc", [2, 128, 24, 128], BF, kind="ExternalInput")
    w1_d = nc.dram_tensor("w1", [2, 128, 8, 128], BF, kind="ExternalInput")
    ud_d = nc.dram_tensor("ud", [2, 128, 48, 128], BF, kind="ExternalInput")
    bias_d = nc.dram_tensor("bias", [128, 12], dt.float32, kind="ExternalInput")
    # outputs declared row-parity-split: [co, jj, t, w] = [co, 2*jj + t, w]
    out_tl = nc.dram_tensor("out_tl", [256, H // 2, 2, 128], dt.float32,
                            kind="ExternalOutput")
    out_br = nc.dram_tensor("out_br", [256, H // 2, 2, 128], dt.float32,
                            kind="ExternalOutput")
    outs = [out_tl, out_br]

    with tile.TileContext(nc) as tc:
        with contextlib.ExitStack() as ctx:
            xpool = ctx.enter_context(tc.tile_pool(name="xp", bufs=1))
            upool = ctx.enter_context(tc.tile_pool(name="up", bufs=1))
            rpool = ctx.enter_context(tc.tile_pool(name="rp", bufs=1))
            vpool = ctx.enter_context(tc.tile_pool(name="vp", bufs=1))
            tpool = ctx.enter_context(tc.tile_pool(name="tp", bufs=2))
            ipool = ctx.enter_context(tc.tile_pool(name="ip", bufs=3))
            opool = ctx.enter_context(tc.tile_pool(name="op", bufs=2))
            cpool = ctx.enter_context(tc.tile_pool(name="cp", bufs=2))
            mpool = ctx.enter_context(tc.tile_pool(name="mp", bufs=1))
            pspool = ctx.enter_context(tc.tile_pool(name="ps", bufs=2,
                                                    space="PSUM"))

            # ---------------- preamble: x, weights, rings ----------------
            xt = [xpool.tile([128, HP, 130], BF, tag=f"x{c}", name=f"x{c}")
                  for c in (0, 1)]
            for c in (0, 1):
                h2 = HP // 2
                nc.sync.dma_start(xt[c][:, h2:, :], xpad.ap()[c][:, h2:, :])
                nc.scalar.dma_start(xt[c][:, :h2, :], xpad.ap()[c][:, :h2, :])

            bt = mpool.tile([128, 12], dt.float32, tag="bias")
            nc.sync.dma_start(bt[:], bias_d.ap())

            # persistent ring tiles (10 rows = 8 + 2 halo), zeroed once
            st = [rpool.tile([128, 10, 130], BF, tag=f"s{r}", name=f"st{r}")
                  for r in range(3)]
            tlt = [[rpool.tile([128, 10, 130], BF, tag=f"t{co}{r}",
                               name=f"tlt{co}{r}")
                    for r in range(3)] for co in range(2)]
            for tile_ in st + tlt[0] + tlt[1]:
                nc.vector.memset(tile_[:].bitcast(dt.uint16), 0.0)

            # parity views: rows (5,2): [p, j, par, w]
            def par(t10):
                return t10[:].rearrange("p (j t) w -> p j t w", t=2)

            xr = [t[:].rearrange("p (j t) w -> p j t w", t=2) for t in xt]

            def inv_group(mg, brow, dst_e, dst_o):
                """ACT evacuates the 4-bank M group in one copy to bf16 SBUF;
                DVE combines (2x); ACT applies bias+ReLU to both parities."""
                s = ipool.tile([128, 4, 512], BF, tag="ev", name="ev")
                for u in range(4):
                    nc.scalar.copy(s[:, u], mg[u][:])
                a = ipool.tile([128, 512], BF, tag="cmb0", name="cmb0")
                b = ipool.tile([128, 512], BF, tag="cmb1", name="cmb1")
                nc.vector.tensor_tensor(a[:], s[:, 0], s[:, 1], Alu.add)
                nc.vector.tensor_tensor(a[:], a[:], s[:, 2], Alu.add)
                nc.vector.tensor_tensor(b[:], s[:, 1], s[:, 2], Alu.subtract)
                nc.vector.tensor_tensor(b[:], b[:], s[:, 3], Alu.subtract)
                r = lambda t: t[:].rearrange("p (a b) -> p a b", b=128)
                nc.scalar.activation(dst_e, r(a), Act.Relu,
                                     bias=bt[:, brow:brow + 1], scale=1.0)
                nc.scalar.activation(dst_o, r(b), Act.Relu,
                                     bias=bt[:, brow:brow + 1], scale=1.0)

            def vmaps(src_par, j0, pfx, n=4):
                """4 V tiles [128, n, 130] from parity view at tile row j0."""
                vt = [vpool.tile([128, 4, 130], BF, tag=f"v{pfx}{u}",
                                 name=f"v{pfx}{u}")
                      for u in range(4)]
                e0 = src_par[:, j0:j0 + n, 0, :]
                o0 = src_par[:, j0:j0 + n, 1, :]
                e1 = src_par[:, j0 + 1:j0 + n + 1, 0, :]
                o1 = src_par[:, j0 + 1:j0 + n + 1, 1, :]
                nc.vector.tensor_tensor(vt[0][:, :n], e0, e1, Alu.subtract)
                nc.vector.tensor_tensor(vt[1][:, :n], o0, e1, Alu.add)
                nc.vector.tensor_tensor(vt[2][:, :n], e1, o0, Alu.subtract)
                nc.vector.tensor_tensor(vt[3][:, :n], o0, o1, Alu.subtract)
                return vt

            def mgroup():
                g = [pspool.tile([128, 512], dt.float32, tag=f"m{u}",
                                 name=f"m{u}")
                     for u in range(4)]
                return g, g

            def wino_mms(mm, vt, w, off, start, stop_banks):
                """Accumulate 4 banks: mm[u] += sum_dx w[:, off+u*3+dx]^T vt[u]."""
                for u in range(4):
                    for dx in range(3):
                        nc.tensor.matmul(
                            mm[u][:], w[:, off + u * 3 + dx],
                            vt[u][:, :, dx:dx + 128],
                            start=start and dx == 0,
                            stop=(u in stop_banks) and dx == 2)

            # ------------------------- branch pass -----------------------
            def branch(bi, desc):
                a_t, a_l = (0, 1) if bi == 0 else (2, 3)
                order = list(reversed(range(CH))) if desc else list(range(CH))
                uat = upool.tile([128, 24, 128], BF, tag="uat", name="uat")
                ual = upool.tile([128, 24, 128], BF, tag="ual", name="ual")
                ucb = upool.tile([128, 24, 128], BF, tag="ucb", name="ucb")
                w1b = upool.tile([128, 8, 128], BF, tag="w1b", name="w1b")
                udb = upool.tile([128, 48, 128], BF, tag="udb", name="udb")
                nc.gpsimd.dma_start(uat[:], ua_d.ap()[a_t])
                nc.gpsimd.dma_start(ual[:], ua_d.ap()[a_l])
                nc.gpsimd.dma_start(ucb[:], uc_d.ap()[bi])
                nc.gpsimd.dma_start(w1b[:], w1_d.ap()[bi])
                nc.gpsimd.dma_start(udb[:], ud_d.ap()[bi])
                ua2 = {a_t: uat, a_l: ual}
                carry = cpool.tile([128, 1, 128], BF, tag=f"cr{bi}")
                nc.vector.memset(carry[:].bitcast(dt.uint16), 0.0)

                def stageA(k, p):
                    nonlocal carry
                    # V of x for this chunk: padded rows 8k..8k+9 -> j0 = 4k
                    vx = [vmaps(xr[c], 4 * k, f"x{c}") for c in (0, 1)]
                    tl_tiles = []
                    for conv in (a_t, a_l):
                        mm, mg = mgroup()
                        for c in (0, 1):
                            wino_mms(mm, vx[c], ua2[conv], 12 * c,
                                     start=c == 0,
                                     stop_banks=(0, 1, 2, 3) if c == 1 else ())
                        tt = tpool.tile([128, 8, 130], BF, tag=f"ab{conv % 2}")
                        tpar = tt[:].rearrange("p (j t) w -> p j t w", t=2)
                        inv_group(mg, conv, tpar[:, 0:4, 0, 1:129],
                                  tpar[:, 0:4, 1, 1:129])
                        tl_tiles.append(tt)
                    tt, lt = tl_tiles
                    # H pool on tt: in-place shifted maxes (DVE streaming)
                    ti = tt[:, :, 1:129]
                    if desc:
                        nc.vector.tensor_tensor(ti[:, 0:7], ti[:, 0:7],
                                                ti[:, 1:8], Alu.max)
                        nc.vector.tensor_tensor(ti[:, 0:6], ti[:, 0:6],
                                                ti[:, 2:8], Alu.max)
                        nc.vector.tensor_tensor(ti[:, 0:4], ti[:, 0:4],
                                                ti[:, 4:8], Alu.max)
                    else:
                        nc.vector.tensor_tensor(ti[:, 1:8], ti[:, 1:8],
                                                ti[:, 0:7], Alu.max)
                        nc.vector.tensor_tensor(ti[:, 2:8], ti[:, 2:8],
                                                ti[:, 0:6], Alu.max)
                        nc.vector.tensor_tensor(ti[:, 4:8], ti[:, 4:8],
                                                ti[:, 0:4], Alu.max)
                    nc.vector.tensor_tensor(ti[:], ti[:],
                                            carry[:].broadcast_to([128, 8, 128]),
                                            Alu.max)
                    if p != CH - 1:
                        nxt = cpool.tile([128, 1, 128], BF, tag=f"cr{bi}")
                        csrc = ti[:, 0:1] if desc else ti[:, 7:8]
                        nc.vector.tensor_copy(nxt[:], csrc)
                        carry = nxt
                    # W pool on lt (reverse for TL, forward for BR)
                    for h in range(8):
                        v = lt[:, h, 1:129]
                        if bi == 0:
                            v = v[:, ::-1]
                        nc.vector.tensor_tensor_scan(v, v, v, 0.0,
                                                     op0=Alu.max, op1=Alu.bypass)
                    # sum -> ring tile k (rows 1..8 interior)
                    s = st[k % 3]
                    nc.vector.tensor_tensor(s[:, 1:9, 1:129], ti[:],
                                            lt[:, :, 1:129], Alu.add)
                    # halo exchange with previously-produced neighbor
                    nb = k + 1 if desc else k - 1
                    if p == 0:
                        edge = s[:, 9:10, :] if desc else s[:, 0:1, :]
                        nc.gpsimd.memset(edge.bitcast(dt.uint16), 0.0)
                    else:
                        nbt = st[nb % 3]
                        if desc:  # my row8 -> nb row0 ; nb row1 -> my row9
                            nc.vector.tensor_copy(nbt[:, 0:1, :], s[:, 8:9, :])
                            nc.vector.tensor_copy(s[:, 9:10, :], nbt[:, 1:2, :])
                        else:     # my row1 -> nb row9 ; nb row8 -> my row0
                            nc.vector.tensor_copy(nbt[:, 9:10, :], s[:, 1:2, :])
                            nc.vector.tensor_copy(s[:, 0:1, :], nbt[:, 8:9, :])
                    if p == CH - 1:
                        edge = s[:, 0:1, :] if desc else s[:, 9:10, :]
                        nc.gpsimd.memset(edge.bitcast(dt.uint16), 0.0)

                def stageC(j):
                    s = st[j % 3]
                    vs = vmaps(par(s), 0, "s")
                    for co in range(2):
                        mm, mg = mgroup()
                        wino_mms(mm, vs, ucb, 12 * co,
                                 start=True, stop_banks=(1, 2))
                        # fold C1: +w1 into m0 (even), -w1 into m3 (odd)
                        for c in (0, 1):
                            nc.tensor.matmul(
                                mm[0][:], w1b[:, co * 4 + 0 * 2 + c],
                                xr[c][:, 4 * j:4 * j + 4, 1, 1:129],
                                start=False, stop=c == 1)
                            nc.tensor.matmul(
                                mm[3][:], w1b[:, co * 4 + 1 * 2 + c],
                                xr[c][:, 4 * j + 1:4 * j + 5, 0, 1:129],
                                start=False, stop=c == 1)
                        d = tlt[co][j % 3]
                        dpar = par(d)
                        brow = 4 + bi * 2 + co
                        inv_group(mg, brow, dpar[:, 0:4, 1, 1:129],
                                  dpar[:, 1:5, 0, 1:129])
                    # halo exchange on tl ring
                    nb = j + 1 if desc else j - 1
                    first = (j == order[0])
                    last = (j == order[-1])
                    for co in range(2):
                        d = tlt[co][j % 3]
                        if first:
                            edge = d[:, 9:10, :] if desc else d[:, 0:1, :]
                            nc.gpsimd.memset(edge.bitcast(dt.uint16), 0.0)
                        else:
                            nbt = tlt[co][nb % 3]
                            if desc:
                                nc.vector.tensor_copy(nbt[:, 0:1, :], d[:, 8:9, :])
                                nc.vector.tensor_copy(d[:, 9:10, :], nbt[:, 1:2, :])
                            else:
                                nc.vector.tensor_copy(nbt[:, 9:10, :], d[:, 1:2, :])
                                nc.vector.tensor_copy(d[:, 0:1, :], nbt[:, 8:9, :])
                        if last:
                            edge = d[:, 0:1, :] if desc else d[:, 9:10, :]
                            nc.gpsimd.memset(edge.bitcast(dt.uint16), 0.0)

                def stageD(j):
                    vt = [vmaps(par(tlt[c][j % 3]), 0, f"d{c}") for c in (0, 1)]
                    orr = outs[bi].ap()
                    for co in range(2):
                        mm, mg = mgroup()
                        for c in (0, 1):
                            wino_mms(mm, vt[c], udb, 24 * co + 12 * c,
                                     start=c == 0,
                                     stop_banks=(0, 1, 2, 3) if c == 1 else ())
                        brow = 8 + bi * 2 + co
                        oe = opool.tile([128, 4, 128], dt.float32, tag="oe",
                                        name="oe")
                        oo = opool.tile([128, 4, 128], dt.float32, tag="oo",
                                        name="oo")
                        inv_group(mg, brow, oe[:], oo[:])
                        for parity, ot in ((0, oe), (1, oo)):
                            nc.sync.dma_start(
                                orr[co * 128:(co + 1) * 128,
                                    4 * j:4 * j + 4, parity, :], ot[:])

                for p, k in enumerate(order):
                    stageA(k, p)
                    if p >= 1:
                        stageC(order[p - 1])
                    if p >= 2:
                        stageD(order[p - 2])
                stageC(order[-1])
                stageD(order[-2])
                stageD(order[-1])

            branch(0, desc=True)
            branch(1, desc=False)

    nc.compile()
    return nc


_NC_CACHE = {}


def _get_nc(H):
    if H not in _NC_CACHE:
        _NC_CACHE[H] = _build(H)
    return _NC_CACHE[H]


def kernel(**inputs):
    from concourse import bass_utils

    x = np.asarray(inputs["x"], np.float32)
    B, C, H, W = x.shape
    assert (C, W) == (256, 128) and H % 8 == 0

    shared = _prep_host(inputs)
    nc = _get_nc(H)

    in_maps = []
    for b in range(B):
        m = dict(shared)
        m["xpad"] = _pad_x_sample(x[b], H)
        in_maps.append(m)

    import os
    trace = bool(int(os.environ.get("KERNEL_TRACE", "0")))
    res = bass_utils.run_bass_kernel_spmd(
        nc, in_maps, core_ids=list(range(B)), trace=trace)
    kernel.last_result = res

    otl = np.stack([res.results[b]["out_tl"].reshape(256, H, 128)
                    for b in range(B)])
    obr = np.stack([res.results[b]["out_br"].reshape(256, H, 128)
                    for b in range(B)])
    return otl, obr


# revision 3
# speedup vs baseline: 1.0165x; 1.0158x over previous
"""CornerPool kernel for Trainium2 — fused 1D Winograd F(2,3) along H, bf16.

One sample per NeuronCore (B=8). All 3x3 convs use Winograd F(2,3) on the
H axis (2 output rows per tile, taps along W stay direct): per output
chunk of 8 rows, 4 PSUM banks accumulate M_u = sum_{ci,dx} U_u^T V_u with
U_u = G-transformed (BN-folded) weights; DVE combines y_even=M0+M1+M2,
y_odd=M1-M2-M3; ScalarE applies bias+ReLU. The 1x1 convs of stage C are
folded into the M0 (+w1) and M3 (-w1) accumulations, so they ride the
same inverse. Corner pools: H pools via shifted-max doubling (GpSimd) +
carry; W pools via DVE prefix-scan per row. The whole net runs fused in
SBUF (two directional passes: TL descending, BR ascending) — x, weights
and rolling sum/tl windows stay on-chip; only x/weights in and outputs
out touch DRAM.
"""

import numpy as np

_P = 128
_CH = 16          # chunks per image; chunk = 8 image rows = 4 Winograd tiles
_G = np.array([[1, 0, 0], [0.5, 0.5, 0.5], [0.5, -0.5, 0.5], [0, 0, 1]],
              np.float32)


def _bf16():
    import ml_dtypes
    return ml_dtypes.bfloat16


def _prep_host(inputs):
    """Fold BN scales, G-transform weights along dy, build bf16 lhsT arrays."""
    f32 = np.float32
    BF = _bf16()

    def scaled(name):
        w = np.asarray(inputs["w_" + name], f32)
        s = np.asarray(inputs["s_" + name], f32)
        return w * s[:, None, None, None]

    def bias(name):
        return np.asarray(inputs["b_" + name], f32)

    def gtrans(w):
        # w [co, ci, 3, 3] -> [ci, 4u, 3dx, co]
        return np.einsum('uy,oiyx->iuxo', _G, w).astype(f32)

    # stage A: [ci=256, 4, 3, co=128] -> [4conv][128k, 2ci*12, 128m]
    def layA(w):
        a = gtrans(w).reshape(2, 128, 12, 128)
        return np.ascontiguousarray(a.transpose(1, 0, 2, 3).reshape(128, 24, 128))

    ua = np.stack([layA(scaled(n)) for n in ("t", "l", "b", "r")]).astype(BF)

    # stage C3: [ci=128, 4, 3, co=256] -> [2br][128k, 2co*12, 128m]
    def layC(w3):
        a = gtrans(w3).reshape(128, 12, 2, 128)
        return np.ascontiguousarray(a.transpose(0, 2, 1, 3).reshape(128, 24, 128))

    uc = np.stack([layC(scaled("tl3")), layC(scaled("br3"))]).astype(BF)

    # stage C1: [co=256, ci=256] -> [2br][128k, co_t*4 + sign*2 + ci_t, 128m]
    def layC1(w1):
        a = w1[:, :, 0, 0].T.reshape(2, 128, 2, 128)   # ci_t, k, co_t, m
        both = np.stack([a, -a], axis=0)               # sign, ci_t, k, co_t, m
        return np.ascontiguousarray(
            both.transpose(2, 3, 0, 1, 4).reshape(128, 8, 128))

    w1 = np.stack([layC1(scaled("tl1")), layC1(scaled("br1"))]).astype(BF)

    # stage D: [ci=256, 4, 3, co=256] -> [2br][128k, co_t*24 + ci_t*12 + uxdx, 128m]
    def layD(w):
        a = gtrans(w).reshape(2, 128, 12, 2, 128)      # ci_t, k, uxdx, co_t, m
        return np.ascontiguousarray(
            a.transpose(1, 3, 0, 2, 4).reshape(128, 48, 128))

    ud = np.stack([layD(scaled("tlo")), layD(scaled("bro"))]).astype(BF)

    bias_rows = [bias("t"), bias("l"), bias("b"), bias("r")]
    for n3, n1 in (("tl3", "tl1"), ("br3", "br1")):
        comb = bias(n3) + bias(n1)
        bias_rows += [comb[:128], comb[128:]]
    for n in ("tlo", "bro"):
        bb = bias(n)
        bias_rows += [bb[:128], bb[128:]]
    bias_all = np.ascontiguousarray(np.stack(bias_rows).T).astype(f32)

    return {"ua": ua, "uc": uc, "w1": w1, "ud": ud, "bias": bias_all}


def _pad_x_sample(xs, H):
    """[256, H, 128] f32 -> [2, 128, H+2, 130] bf16 zero-padded."""
    BF = _bf16()
    xp = np.zeros((2, 128, H + 2, 130), BF)
    xp[:, :, 1:H + 1, 1:129] = xs.reshape(2, 128, H, 128).astype(BF)
    return xp


def _build(H):
    import concourse.bacc as bacc
    import concourse.mybir as mybir
    import concourse.tile as tile
    import contextlib

    dt = mybir.dt
    Alu = mybir.AluOpType
    Act = mybir.ActivationFunctionType
    BF = dt.bfloat16
    CH = H // 8
    HP = H + 2

    nc = bacc.Bacc("TRN2", target_bir_lowering=False, debug=False)

    xpad = nc.dram_tensor("xpad", [2, 128, HP, 130], BF, kind="ExternalInput")
    ua_d = nc.dram_tensor("ua", [4, 128, 24, 128], BF, kind="ExternalInput")
    uc_d = nc.dram_tensor("uc", [2, 128, 24, 128], BF, kind="ExternalInput")
    w1_d = nc.dram_tensor("w1", [2, 128, 8, 128], BF, kind="ExternalInput")
    ud_d = nc.dram_tensor("ud", [2, 128, 48, 128], BF, kind="ExternalInput")
    bias_d = nc.dram_tensor("bias", [128, 12], dt.float32, kind="ExternalInput")
    # outputs declared row-parity-split: [co, jj, t, w] = [co, 2*jj + t, w]
    out_tl = nc.dram_tensor("out_tl", [256, H // 2, 2, 128], dt.float32,
                            kind="ExternalOutput")
    out_br = nc.dram_tensor("out_br", [256, H // 2, 2, 128], dt.float32,
                            kind="ExternalOutput")
    outs = [out_tl, out_br]

    with tile.TileContext(nc) as tc:
        with contextlib.ExitStack() as ctx:
            xpool = ctx.enter_context(tc.tile_pool(name="xp", bufs=1))
            upool = ctx.enter_context(tc.tile_pool(name="up", bufs=1))
            rpool = ctx.enter_context(tc.tile_pool(name="rp", bufs=1))
            vpool = ctx.enter_context(tc.tile_pool(name="vp", bufs=1))
            tpool = ctx.enter_context(tc.tile_pool(name="tp", bufs=2))
            ipool = ctx.enter_context(tc.tile_pool(name="ip", bufs=3))
            opool = ctx.enter_context(tc.tile_pool(name="op", bufs=2))
            cpool = ctx.enter_context(tc.tile_pool(name="cp", bufs=2))
            mpool = ctx.enter_context(tc.tile_pool(name="mp", bufs=1))
            pspool = ctx.enter_context(tc.tile_pool(name="ps", bufs=2,
                                                    space="PSUM"))

            # ---------------- preamble: x, weights, rings ----------------
            xt = [xpool.tile([128, HP, 130], BF, tag=f"x{c}", name=f"x{c}")
                  for c in (0, 1)]
            bt = mpool.tile([128, 12], dt.float32, tag="bias")
            nc.gpsimd.dma_start(bt[:], bias_d.ap())

            def load_wb(bi):
                """Load one branch's transformed weights into SBUF."""
                a_t, a_l = (0, 1) if bi == 0 else (2, 3)
                uat = upool.tile([128, 24, 128], BF, tag="uat", name="uat",
                                 bufs=2)
                ual = upool.tile([128, 24, 128], BF, tag="ual", name="ual",
                                 bufs=2)
                ucb = upool.tile([128, 24, 128], BF, tag="ucb", name="ucb")
                w1b = upool.tile([128, 8, 128], BF, tag="w1b", name="w1b")
                udb = upool.tile([128, 48, 128], BF, tag="udb", name="udb")
                nc.gpsimd.dma_start(uat[:], ua_d.ap()[a_t])
                nc.gpsimd.dma_start(ual[:], ua_d.ap()[a_l])
                nc.gpsimd.dma_start(ucb[:], uc_d.ap()[bi])
                nc.gpsimd.dma_start(w1b[:], w1_d.ap()[bi])
                nc.gpsimd.dma_start(udb[:], ud_d.ap()[bi])
                return {a_t: uat, a_l: ual}, ucb, w1b, udb

            # x lands in need-order for the first (descending) branch:
            # top 10 rows (first chunk) -> stage-A weights -> the rest in
            # progressively larger slices. The DMA pool drains serially, so
            # byte order here is the PE-start latency.
            for c in (0, 1):
                eng = nc.sync if c == 0 else nc.scalar
                eng.dma_start(xt[c][:, HP - 10:, :], xpad.ap()[c][:, HP - 10:, :])
            wb0 = load_wb(0)
            cuts = [HP - 10, HP - 42, HP - 74, 0]
            for i in range(len(cuts) - 1):
                a, b = cuts[i + 1], cuts[i]
                for c in (0, 1):
                    eng = nc.sync if c == 0 else nc.scalar
                    eng.dma_start(xt[c][:, a:b, :], xpad.ap()[c][:, a:b, :])

            # persistent ring tiles (10 rows = 8 + 2 halo), zeroed once
            st = [rpool.tile([128, 10, 130], BF, tag=f"s{r}", name=f"st{r}")
                  for r in range(3)]
            tlt = [[rpool.tile([128, 10, 130], BF, tag=f"t{co}{r}",
                               name=f"tlt{co}{r}")
                    for r in range(3)] for co in range(2)]
            for tile_ in st + tlt[0] + tlt[1]:
                nc.vector.memset(tile_[:].bitcast(dt.uint16), 0.0)

            # parity views: rows (5,2): [p, j, par, w]
            def par(t10):
                return t10[:].rearrange("p (j t) w -> p j t w", t=2)

            xr = [t[:].rearrange("p (j t) w -> p j t w", t=2) for t in xt]

            def inv_group(mg, brow, dst_e, dst_o):
                """ACT evacuates the 4-bank M group in one copy to bf16 SBUF;
                DVE combines (2x); ACT applies bias+ReLU to both parities."""
                s = ipool.tile([128, 4, 512], BF, tag="ev", name="ev")
                for u in range(4):
                    nc.scalar.copy(s[:, u], mg[u][:])
                a = ipool.tile([128, 512], BF, tag="cmb0", name="cmb0")
                b = ipool.tile([128, 512], BF, tag="cmb1", name="cmb1")
                nc.vector.tensor_tensor(a[:], s[:, 0], s[:, 1], Alu.add)
                nc.vector.tensor_tensor(a[:], a[:], s[:, 2], Alu.add)
                nc.vector.tensor_tensor(b[:], s[:, 1], s[:, 2], Alu.subtract)
                nc.vector.tensor_tensor(b[:], b[:], s[:, 3], Alu.subtract)
                r = lambda t: t[:].rearrange("p (a b) -> p a b", b=128)
                nc.scalar.activation(dst_e, r(a), Act.Relu,
                                     bias=bt[:, brow:brow + 1], scale=1.0)
                nc.scalar.activation(dst_o, r(b), Act.Relu,
                                     bias=bt[:, brow:brow + 1], scale=1.0)

            def vmaps(src_par, j0, pfx, n=4):
                """4 V tiles [128, n, 130] from parity view at tile row j0."""
                vt = [vpool.tile([128, 4, 130], BF, tag=f"v{pfx}{u}",
                                 name=f"v{pfx}{u}")
                      for u in range(4)]
                e0 = src_par[:, j0:j0 + n, 0, :]
                o0 = src_par[:, j0:j0 + n, 1, :]
                e1 = src_par[:, j0 + 1:j0 + n + 1, 0, :]
                o1 = src_par[:, j0 + 1:j0 + n + 1, 1, :]
                nc.vector.tensor_tensor(vt[0][:, :n], e0, e1, Alu.subtract)
                nc.vector.tensor_tensor(vt[1][:, :n], o0, e1, Alu.add)
                nc.vector.tensor_tensor(vt[2][:, :n], e1, o0, Alu.subtract)
                nc.vector.tensor_tensor(vt[3][:, :n], o0, o1, Alu.subtract)
                return vt

            def mgroup():
                g = [pspool.tile([128, 512], dt.float32, tag=f"m{u}",
                                 name=f"m{u}")
                     for u in range(4)]
                return g, g

            def wino_mms(mm, vt, w, off, start, stop_banks):
                """Accumulate 4 banks: mm[u] += sum_dx w[:, off+u*3+dx]^T vt[u]."""
                for u in range(4):
                    for dx in range(3):
                        nc.tensor.matmul(
                            mm[u][:], w[:, off + u * 3 + dx],
                            vt[u][:, :, dx:dx + 128],
                            start=start and dx == 0,
                            stop=(u in stop_banks) and dx == 2)

            # ------------------------- branch pass -----------------------
            def branch(bi, desc):
                a_t, a_l = (0, 1) if bi == 0 else (2, 3)
                order = list(reversed(range(CH))) if desc else list(range(CH))
                ua2, ucb, w1b, udb = wb0 if bi == 0 else load_wb(1)
                carry = cpool.tile([128, 1, 128], BF, tag=f"cr{bi}")
                nc.vector.memset(carry[:].bitcast(dt.uint16), 0.0)

                def stageA(k, p):
                    nonlocal carry
                    # V of x for this chunk: padded rows 8k..8k+9 -> j0 = 4k
                    vx = [vmaps(xr[c], 4 * k, f"x{c}") for c in (0, 1)]
                    tl_tiles = []
                    for conv in (a_t, a_l):
                        mm, mg = mgroup()
                        for c in (0, 1):
                            wino_mms(mm, vx[c], ua2[conv], 12 * c,
                                     start=c == 0,
                                     stop_banks=(0, 1, 2, 3) if c == 1 else ())
                        tt = tpool.tile([128, 8, 130], BF, tag=f"ab{conv % 2}")
                        tpar = tt[:].rearrange("p (j t) w -> p j t w", t=2)
                        inv_group(mg, conv, tpar[:, 0:4, 0, 1:129],
                                  tpar[:, 0:4, 1, 1:129])
                        tl_tiles.append(tt)
                    tt, lt = tl_tiles
                    # H pool on tt: in-place shifted maxes (DVE streaming)
                    ti = tt[:, :, 1:129]
                    if desc:
                        nc.vector.tensor_tensor(ti[:, 0:7], ti[:, 0:7],
                                                ti[:, 1:8], Alu.max)
                        nc.vector.tensor_tensor(ti[:, 0:6], ti[:, 0:6],
                                                ti[:, 2:8], Alu.max)
                        nc.vector.tensor_tensor(ti[:, 0:4], ti[:, 0:4],
                                                ti[:, 4:8], Alu.max)
                    else:
                        nc.vector.tensor_tensor(ti[:, 1:8], ti[:, 1:8],
                                                ti[:, 0:7], Alu.max)
                        nc.vector.tensor_tensor(ti[:, 2:8], ti[:, 2:8],
                                                ti[:, 0:6], Alu.max)
                        nc.vector.tensor_tensor(ti[:, 4:8], ti[:, 4:8],
                                                ti[:, 0:4], Alu.max)
                    nc.vector.tensor_tensor(ti[:], ti[:],
                                            carry[:].broadcast_to([128, 8, 128]),
                                            Alu.max)
                    if p != CH - 1:
                        nxt = cpool.tile([128, 1, 128], BF, tag=f"cr{bi}")
                        csrc = ti[:, 0:1] if desc else ti[:, 7:8]
                        nc.vector.tensor_copy(nxt[:], csrc)
                        carry = nxt
                    # W pool on lt (reverse for TL, forward for BR)
                    for h in range(8):
                        v = lt[:, h, 1:129]
                        if bi == 0:
                            v = v[:, ::-1]
                        nc.vector.tensor_tensor_scan(v, v, v, 0.0,
                                                     op0=Alu.max, op1=Alu.bypass)
                    # sum -> ring tile k (rows 1..8 interior)
                    s = st[k % 3]
                    nc.vector.tensor_tensor(s[:, 1:9, 1:129], ti[:],
                                            lt[:, :, 1:129], Alu.add)
                    # halo exchange with previously-produced neighbor
                    nb = k + 1 if desc else k - 1
                    if p == 0:
                        edge = s[:, 9:10, :] if desc else s[:, 0:1, :]
                        nc.gpsimd.memset(edge.bitcast(dt.uint16), 0.0)
                    else:
                        nbt = st[nb % 3]
                        if desc:  # my row8 -> nb row0 ; nb row1 -> my row9
                            nc.vector.tensor_copy(nbt[:, 0:1, :], s[:, 8:9, :])
                            nc.vector.tensor_copy(s[:, 9:10, :], nbt[:, 1:2, :])
                        else:     # my row1 -> nb row9 ; nb row8 -> my row0
                            nc.vector.tensor_copy(nbt[:, 9:10, :], s[:, 1:2, :])
                            nc.vector.tensor_copy(s[:, 0:1, :], nbt[:, 8:9, :])
                    if p == CH - 1:
                        edge = s[:, 0:1, :] if desc else s[:, 9:10, :]
                        nc.gpsimd.memset(edge.bitcast(dt.uint16), 0.0)

                def stageC(j):
                    s = st[j % 3]
                    vs = vmaps(par(s), 0, "s")
                    for co in range(2):
                        mm, mg = mgroup()
                        wino_mms(mm, vs, ucb, 12 * co,
                                 start=True, stop_banks=(1, 2))
                        # fold C1: +w1 into m0 (even), -w1 into m3 (odd)
                        for c in (0, 1):
                            nc.tensor.matmul(
                                mm[0][:], w1b[:, co * 4 + 0 * 2 + c],
                                xr[c][:, 4 * j:4 * j + 4, 1, 1:129],
                                start=False, stop=c == 1)
                            nc.tensor.matmul(
                                mm[3][:], w1b[:, co * 4 + 1 * 2 + c],
                                xr[c][:, 4 * j + 1:4 * j + 5, 0, 1:129],
                                start=False, stop=c == 1)
                        d = tlt[co][j % 3]
                        dpar = par(d)
                        brow = 4 + bi * 2 + co
                        inv_group(mg, brow, dpar[:, 0:4, 1, 1:129],
                                  dpar[:, 1:5, 0, 1:129])
                    # halo exchange on tl ring
                    nb = j + 1 if desc else j - 1
                    first = (j == order[0])
                    last = (j == order[-1])
                    for co in range(2):
                        d = tlt[co][j % 3]
                        if first:
                            edge = d[:, 9:10, :] if desc else d[:, 0:1, :]
                            nc.gpsimd.memset(edge.bitcast(dt.uint16), 0.0)
                        else:
                            nbt = tlt[co][nb % 3]
                            if desc:
                                nc.vector.tensor_copy(nbt[:, 0:1, :], d[:, 8:9, :])
                                nc.vector.tensor_copy(d[:, 9:10, :], nbt[:, 1:2, :])
                            else:
                                nc.vector.tensor_copy(nbt[:, 9:10, :], d[:, 1:2, :])
                                nc.vector.tensor_copy(d[:, 0:1, :], nbt[:, 8:9, :])
                        if last:
                            edge = d[:, 0:1, :] if desc else d[:, 9:10, :]
                            nc.gpsimd.memset(edge.bitcast(dt.uint16), 0.0)

                def stageD(j):
                    vt = [vmaps(par(tlt[c][j % 3]), 0, f"d{c}") for c in (0, 1)]
                    orr = outs[bi].ap()
                    for co in range(2):
                        mm, mg = mgroup()
                        for c in (0, 1):
                            wino_mms(mm, vt[c], udb, 24 * co + 12 * c,
                                     start=c == 0,
                                     stop_banks=(0, 1, 2, 3) if c == 1 else ())
                        brow = 8 + bi * 2 + co
                        oe = opool.tile([128, 4, 128], dt.float32, tag="oe",
                                        name="oe")
                        oo = opool.tile([128, 4, 128], dt.float32, tag="oo",
                                        name="oo")
                        inv_group(mg, brow, oe[:], oo[:])
                        for parity, ot in ((0, oe), (1, oo)):
                            nc.sync.dma_start(
                                orr[co * 128:(co + 1) * 128,
                                    4 * j:4 * j + 4, parity, :], ot[:])

                for p, k in enumerate(order):
                    stageA(k, p)
                    if p >= 1:
                        stageC(order[p - 1])
                    if p >= 2:
                        stageD(order[p - 2])
                stageC(order[-1])
                stageD(order[-2])
                stageD(order[-1])

            branch(0, desc=True)
            branch(1, desc=False)

    nc.compile()
    return nc


_NC_CACHE = {}


def _get_nc(H):
    if H not in _NC_CACHE:
        _NC_CACHE[H] = _build(H)
    return _NC_CACHE[H]


def kernel(**inputs):
    from concourse import bass_utils

    x = np.asarray(inputs["x"], np.float32)
    B, C, H, W = x.shape
    assert (C, W) == (256, 128) and H % 8 == 0

    shared = _prep_host(inputs)
    nc = _get_nc(H)

    in_maps = []
    for b in range(B):
        m = dict(shared)
        m["xpad"] = _pad_x_sample(x[b], H)
        in_maps.append(m)

    import os
    trace = bool(int(os.environ.get("KERNEL_TRACE", "0")))
    res = bass_utils.run_bass_kernel_spmd(
        nc, in_maps, core_ids=list(range(B)), trace=trace)
    kernel.last_result = res

    otl = np.stack([res.results[b]["out_tl"].reshape(256, H, 128)
                    for b in range(B)])
    obr = np.stack([res.results[b]["out_br"].reshape(256, H, 128)
                    for b in range(B)])
    return otl, obr
